# revision 51
# baseline (speedup 1.0000x reference)
"""Dense soft-MoE (ExpertAdapter) Trainium2 Bass kernel.

Reference computation (per token t):
    r = softmax(x @ Wr + br)                      # [E]
    h_e = gelu(x @ W1[e] + b1[e])                 # [F]
    y_e = h_e @ W2[e] + b2[e]                     # [D]
    out = sum_e r[e] * y_e                        # [D]

Strategy: pure data-parallel over tokens across 8 NeuronCores (weights are
replicated).  Per core (2048 tokens) everything is computed in fp16 matmuls
with fp32 PSUM accumulation:

  out[t,:] = rinv[t] * ( sum_e sum_f (gelu(hT) * exp_e)[f,t] W2[e][f,:]
                         + sum_e exp[e,t] b2[e,:] )
  where exp[e,t] = exp(logit[e,t] + br[e]),  rinv[t] = 1/sum_e exp[e,t]

so the expert combine happens *inside* PSUM accumulation (router weights are
folded into the GEMM2 lhs via an unnormalized-exp broadcast; the softmax
denominator is applied once at the end as a per-partition scale).

Layouts (per 256-token "big tile", 2 sub-tiles of 128 tokens):
  xT   [128d, (k=8, tok=256)]  fp16   (PE-transposed from x)
  hT   [128f, tok]             PSUM   (GEMM1: lhsT=W1 chunk, rhs=xT chunk)
  y    [128tok, 1024d]         PSUM   (GEMM2: lhsT=scaled hT chunk, rhs=W2)
"""

import numpy as np

import concourse.bacc as bacc
import concourse.mybir as mybir
import concourse.tile as tile
from concourse.bass_utils import run_bass_kernel_spmd
from concourse.masks import make_identity

# Problem constants (hardcoded per spec nn_ExpertAdapter_13640816132847)
B, N, D, E, F = 4, 4096, 1024, 8, 512
N_CORES = 8
TOK_PER_CORE = (B * N) // N_CORES  # 2048

F16 = mybir.dt.float16
F32 = mybir.dt.float32

KD = D // 128   # 8 d-chunks
FC = F // 128   # 4 f-chunks
BT = 256        # tokens per big tile
NSUB = BT // 128  # 2 sub-tiles


def build_moe(n_tok=TOK_PER_CORE, repeat=1, sim_safe=False):
    """Build the per-core Bass program. Inputs are the full replicated weights
    plus this core's x shard [n_tok, D].

    sim_safe=True replaces Gelu (not implemented in CoreSim) with Identity so
    the dataflow can be validated in simulation; hardware always uses Gelu."""
    assert n_tok % BT == 0
    nbt = n_tok // BT
    gelu_fn = (mybir.ActivationFunctionType.Identity if sim_safe
               else mybir.ActivationFunctionType.Gelu)

    nc = bacc.Bacc("TRN2", target_bir_lowering=False, debug=False)

    x_d = nc.dram_tensor("x", [n_tok, D], F32, kind="ExternalInput").ap()
    w1_d = nc.dram_tensor("W1", [E, D, F], F32, kind="ExternalInput").ap()
    b1_d = nc.dram_tensor("b1", [E, F], F32, kind="ExternalInput").ap()
    w2_d = nc.dram_tensor("W2", [E, F, D], F32, kind="ExternalInput").ap()
    b2_d = nc.dram_tensor("b2", [E, D], F32, kind="ExternalInput").ap()
    wr_d = nc.dram_tensor("Wr", [D, E], F32, kind="ExternalInput").ap()
    br_d = nc.dram_tensor("br", [E], F32, kind="ExternalInput").ap()
    out_d = nc.dram_tensor("out", [n_tok, D], F32, kind="ExternalOutput").ap()

    with tile.TileContext(nc) as tc:
        with (
            tc.tile_pool(name="consts", bufs=1) as consts,
            tc.tile_pool(name="wres", bufs=1) as wres,
            tc.tile_pool(name="wstage", bufs=3) as wstage,
            tc.tile_pool(name="xin", bufs=4) as xin,
            tc.tile_pool(name="xtp", bufs=2) as xtp,
            tc.tile_pool(name="hgp", bufs=2) as hgp,
            tc.tile_pool(name="hsp", bufs=2) as hsp,
            tc.tile_pool(name="routp", bufs=2) as routp,
            tc.tile_pool(name="outp", bufs=2) as outp,
            tc.tile_pool(name="ps_y", bufs=2, space="PSUM") as ps_y,
            tc.tile_pool(name="ps_h", bufs=1, space="PSUM") as ps_h,
            tc.tile_pool(name="ps_t", bufs=2, space="PSUM") as ps_t,
            tc.tile_pool(name="ps_r", bufs=1, space="PSUM") as ps_r,
        ):
            # ---- constants / weights (once) ----
            ident = consts.tile([128, 128], F32)
            make_identity(nc, ident[:])
            ones8 = consts.tile([E, 1], F16)
            nc.gpsimd.memset(ones8[:], 1.0)

            # router weights [128, (k e)] fp16
            wr_st = consts.tile([128, KD * E], F32)
            nc.sync.dma_start(wr_st[:], wr_d.rearrange("(k p) e -> p k e", p=128))
            wr_h = consts.tile([128, KD * E], F16)
            nc.gpsimd.tensor_copy(wr_h[:], wr_st[:])

            br_col = consts.tile([E, 1], F32)
            nc.sync.dma_start(br_col[:, 0:1], br_d[:])

            # b1 as per-partition bias columns [128, (e fc)]
            b1_sb = consts.tile([128, E * FC], F32)
            nc.sync.dma_start(b1_sb[:], b1_d.rearrange("e (c p) -> p e c", p=128))

            # b2 [E, D] fp16 (K=8 matmul operand)
            b2_st = consts.tile([E, D], F32)
            nc.sync.dma_start(b2_st[:], b2_d[:])
            b2_h = consts.tile([E, D], F16)
            nc.gpsimd.tensor_copy(b2_h[:], b2_st[:])

            # expert weights, resident fp16
            # W1h [128, (e k f)]: lhsT chunk for (e,k,fc) at e*KD*F + k*F + fc*128
            # W2h [128, (e kf d)]: rhs for (e,kf,dh) at e*FC*D + kf*D + dh*512
            w1_h = wres.tile([128, E * KD * F], F16)
            w2_h = wres.tile([128, E * KD * F], F16, name="w2_h")
            CH = 1024  # fp32 staging chunk (elems per partition)
            # Pool is otherwise idle; ACT/DVE carry gelu/mult so only get a
            # small share of the weight casts.
            cast_engines = [nc.gpsimd, nc.gpsimd, nc.gpsimd, nc.vector,
                            nc.gpsimd, nc.gpsimd, nc.gpsimd, nc.scalar]
            n_cast = 0

            def _cast(dst, src):
                nonlocal n_cast
                eng = cast_engines[n_cast % len(cast_engines)]
                if eng is nc.scalar:
                    eng.copy(dst, src)
                else:
                    eng.tensor_copy(dst, src)
                n_cast += 1

            def emit_weight_load(e, part="both"):
                """DMA + cast expert e's W1 and/or W2 into the resident fp16
                tiles. GEMM1 needs W1[e] a full pipeline stage before GEMM2
                needs W2[e], so startup emits W1s ahead of W2s."""
                c1 = CH // F
                c2 = CH // D
                if part in ("both", "w1"):
                    w1_r = w1_d[e].rearrange("(k p) f -> p k f", p=128)
                    for c in range(KD * F // CH):
                        st = wstage.tile([128, CH], F32, tag="wst")
                        nc.sync.dma_start(st[:], w1_r[:, c * c1:(c + 1) * c1, :])
                        _cast(w1_h[:, e * KD * F + c * CH:
                                   e * KD * F + (c + 1) * CH], st[:])
                if part in ("both", "w2"):
                    w2_r = w2_d[e].rearrange("(k p) d2 -> p k d2", p=128)
                    for c in range(FC * D // CH):
                        st = wstage.tile([128, CH], F32, tag="wst")
                        nc.sync.dma_start(st[:], w2_r[:, c * c2:(c + 1) * c2, :])
                        _cast(w2_h[:, e * FC * D + c * CH:
                                   e * FC * D + (c + 1) * CH], st[:])

            def prep_xT(xi_tiles):
                """PE-transpose x's [128tok,128d] blocks into fp16
                xT [128d, (k, s, tok)]; PSUM packs of 4, copies split
                across ACT and DVE."""
                xT_t = xtp.tile([128, KD * BT], F16, name="xT")
                for g in range(KD * NSUB // 4):
                    tp = ps_t.tile([128, 512], F32, tag="tp", name="tp")
                    for j in range(4):
                        idx = g * 4 + j
                        k, s = idx // NSUB, idx % NSUB
                        nc.tensor.transpose(
                            tp[:, j * 128:(j + 1) * 128],
                            xi_tiles[s][:, k * 128:(k + 1) * 128], ident[:])
                        dst = xT_t[:, k * BT + s * 128: k * BT + (s + 1) * 128]
                        if j % 2 == 0:
                            nc.scalar.copy(dst, tp[:, j * 128:(j + 1) * 128])
                        else:
                            nc.vector.tensor_copy(dst, tp[:, j * 128:(j + 1) * 128])
                return xT_t

            # big tile 0's x goes on the DMA queue before any weight traffic
            # so the transposes/router can start immediately.
            x_pre = []
            for s in range(NSUB):
                x_in0 = xin.tile([128, D], F32, tag="xin", name="x_in0")
                nc.sync.dma_start(x_in0[:], x_d[s * 128:(s + 1) * 128, :])
                x_pre.append(x_in0)

            # first two experts' W1 up-front (W2 lags one stage thanks to the
            # pipelined GEMM2); the rest are woven into big tile 0's expert
            # loop so early GEMMs aren't starved by casts.
            emit_weight_load(0, "w1")
            emit_weight_load(1, "w1")
            emit_weight_load(0, "w2")
            emit_weight_load(1, "w2")
            weights_loaded = 2

            # ---- main loop ----
            # one-tile pipeline: xT(t) is prepared during tile t-1's compute
            xT = prep_xT(x_pre)
            for rep in range(repeat):
                for t in range(nbt):
                    tok0 = t * BT

                    # prefetch next tile's x + build its xT now, so the DMAs
                    # sit ahead of this tile's out-stores in the queue and the
                    # cast/transposes overlap this tile's compute
                    if t + 1 < nbt or rep + 1 < repeat:
                        nt0 = ((t + 1) % nbt) * BT
                        xi_next = []
                        for s in range(NSUB):
                            x_in = xin.tile([128, D], F32, tag="xin")
                            nc.sync.dma_start(
                                x_in[:], x_d[nt0 + s * 128: nt0 + (s + 1) * 128, :])
                            xi_next.append(x_in)
                        xT_next = prep_xT(xi_next)
                    else:
                        xT_next = None

                    # router: logits^T [E, BT] in PSUM, exp, sums, 1/sums
                    rt = ps_r.tile([128, 512], F32, tag="rt")
                    for k in range(KD):
                        nc.tensor.matmul(
                            rt[0:E, 0:BT],
                            wr_h[:, k * E:(k + 1) * E],
                            xT[:, k * BT:(k + 1) * BT],
                            start=(k == 0), stop=(k == KD - 1))
                    exp_h = routp.tile([E, BT], F16, tag="exp")
                    nc.scalar.activation(exp_h[:], rt[0:E, 0:BT],
                                         mybir.ActivationFunctionType.Exp,
                                         bias=br_col[:], scale=1.0)
                    nc.tensor.matmul(rt[0:1, 256:256 + BT], ones8[:], exp_h[:],
                                     start=True, stop=True)
                    sums_sb = routp.tile([1, BT], F32, tag="sums")
                    nc.scalar.copy(sums_sb[0:1, :], rt[0:1, 256:256 + BT])
                    exp_row = routp.tile([1, E * BT], F16, tag="expr")
                    nc.sync.dma_start(exp_row[0:1, :], exp_h[:])
                    rinvs = []
                    for s in range(NSUB):
                        s_col = routp.tile([128, 1], F32, tag="scol")
                        nc.sync.dma_start(s_col[:, 0:1],
                                          sums_sb[0:1, s * 128:(s + 1) * 128])
                        rinv = routp.tile([128, 1], F32, tag="rinv")
                        nc.vector.reciprocal(rinv[:], s_col[:])
                        rinvs.append(rinv)

                    # expert loop, software-pipelined: GEMM2(e) is emitted
                    # after GEMM1(e+1) so PE never waits on the
                    # gelu->mult chain producing hs.
                    ys = [ps_y.tile([128, D], F32, tag="y", name=f"y{s}")
                          for s in range(NSUB)]
                    hps = ps_h.tile([128, 512], F32, tag="h", name="hps")
                    # all experts' router-weight broadcasts up-front (Pool)
                    wes = []
                    for e in range(E):
                        we = hsp.tile([128, BT], F16, tag="we", name="we", bufs=E)
                        nc.gpsimd.partition_broadcast(
                            we[:], exp_row[0:1, e * BT:(e + 1) * BT])
                        wes.append(we)

                    def emit_gemm2(e, hs_e):
                        for fc in range(FC):
                            for s in range(NSUB):
                                for dh in range(2):
                                    nc.tensor.matmul(
                                        ys[s][:, dh * 512:(dh + 1) * 512],
                                        hs_e[:, fc * BT + s * 128:
                                             fc * BT + (s + 1) * 128],
                                        w2_h[:, e * FC * D + fc * D + dh * 512:
                                             e * FC * D + fc * D + (dh + 1) * 512],
                                        start=False,
                                        stop=(e == E - 1 and fc == FC - 1))

                    # b2 (weighted by exp) opens each PSUM accumulation group
                    # so nothing trails the last expert's GEMM2
                    for s in range(NSUB):
                        for dh in range(2):
                            nc.tensor.matmul(
                                ys[s][:, dh * 512:(dh + 1) * 512],
                                exp_h[:, s * 128:(s + 1) * 128],
                                b2_h[:, dh * 512:(dh + 1) * 512],
                                start=True, stop=False)

                    prev = None
                    for e in range(E):
                        # weave remaining weight loads one expert ahead of use
                        if rep == 0 and t == 0 and weights_loaded < E:
                            emit_weight_load(weights_loaded)
                            weights_loaded += 1
                        hg = hgp.tile([128, FC * BT], F32, tag="hg")
                        hs = hsp.tile([128, FC * BT], F16, tag="hs")
                        we = wes[e]
                        for fc in range(FC):
                            hreg = hps[:, (fc % 2) * BT:(fc % 2) * BT + BT]
                            for k in range(KD):
                                nc.tensor.matmul(
                                    hreg,
                                    w1_h[:, e * KD * F + k * F + fc * 128:
                                         e * KD * F + k * F + (fc + 1) * 128],
                                    xT[:, k * BT:(k + 1) * BT],
                                    start=(k == 0), stop=(k == KD - 1))
                            nc.scalar.activation(
                                hg[:, fc * BT:(fc + 1) * BT], hreg,
                                gelu_fn,
                                bias=b1_sb[:, e * FC + fc: e * FC + fc + 1],
                                scale=1.0)
                            nc.vector.tensor_mul(
                                hs[:, fc * BT:(fc + 1) * BT],
                                hg[:, fc * BT:(fc + 1) * BT], we)
                        if prev is not None:
                            emit_gemm2(*prev)
                        prev = (e, hs)
                    emit_gemm2(*prev)

                    # scale by 1/sums, store
                    for s in range(NSUB):
                        o_sb = outp.tile([128, D], F32, tag="osb")
                        nc.scalar.mul(o_sb[:], ys[s][:], rinvs[s][:])
                        nc.sync.dma_start(
                            out_d[tok0 + s * 128: tok0 + (s + 1) * 128, :], o_sb[:])
                    xT = xT_next

    nc.compile()
    return nc


_NC_CACHE = {}


def _get_nc(n_tok=TOK_PER_CORE, repeat=1):
    key = (n_tok, repeat)
    if key not in _NC_CACHE:
        _NC_CACHE[key] = build_moe(n_tok, repeat)
    return _NC_CACHE[key]


def kernel(x, W1, b1, W2, b2, Wr, br):
    x = np.ascontiguousarray(np.asarray(x, dtype=np.float32))
    W1 = np.ascontiguousarray(np.asarray(W1, dtype=np.float32))
    b1 = np.ascontiguousarray(np.asarray(b1, dtype=np.float32))
    W2 = np.ascontiguousarray(np.asarray(W2, dtype=np.float32))
    b2 = np.ascontiguousarray(np.asarray(b2, dtype=np.float32))
    Wr = np.ascontiguousarray(np.asarray(Wr, dtype=np.float32))
    br = np.ascontiguousarray(np.asarray(br, dtype=np.float32))

    xf = x.reshape(B * N, D)
    nc = _get_nc()
    in_maps = [
        {"x": np.ascontiguousarray(xf[c * TOK_PER_CORE:(c + 1) * TOK_PER_CORE]),
         "W1": W1, "b1": b1, "W2": W2, "b2": b2, "Wr": Wr, "br": br}
        for c in range(N_CORES)
    ]
    try:
        res = run_bass_kernel_spmd(nc, in_maps, core_ids=list(range(N_CORES)))
    except Exception:
        # A previously-wedged NeuronCore clears with a core reset on retry.
        import os
        os.environ.setdefault("NEURON_RT_RESET_CORES", "1")
        res = run_bass_kernel_spmd(nc, in_maps, core_ids=list(range(N_CORES)))
    out = np.concatenate([r["out"] for r in res.results], axis=0)
    return out.reshape(B, N, D).astype(np.float32)
